# revision 34
# baseline (speedup 1.0000x reference)
"""Trainium2 Bass kernel for nn_Attention (B=4, P=2048, D=768, H=12, hd=64).

Sharding: 8 cores = 4 batches x 2 half-head-groups (6 heads each).

Schedule (284us vs 301us baseline): the Scalar Exp stream (25.2M
elems/core, ~1.08us per 1024-col unit, 192 units) is the critical
resource; everything else hides inside it:
  - all input DMA on the single sync ring in strict priority order
    (xT + the ft0/ft3 wqk columns first, then wv, wqk-rest, wp) and a
    cc-outer phase A computing only ft0/ft3 across 8 PSUM banks, so the
    first Exp fires at ~29us instead of ~55us;
  - uniform 1024-col Exp units (16/chunk): PSUM = 2x2 banks scores +
    2 banks AV + 2 banks fills; leftover qk projection (ft1/2/4/5),
    v projection, and output projection stream as paced fill jobs in
    AV-free units instead of serializing before/after the stream;
  - v projection fills chunk 0 and units 0-7 of chunk 1 (AV_0 is
    compressed into chunk 1 units 8-15); AV otherwise spreads over
    units 0-10 so the slab buffer frees before the next chunk reuses it;
  - output projection per token block is appended to the fill queue as
    soon as its 6 heads are normalized, so its compute and yT DMA
    overlap the stream; only the last blocks remain in the tail.

Per-core layouts (host-prepared):
  xT   [769, 2048] bf16  rows 0..767 = x[b].T, row 768 = ones
  wqk  [768, 768]  bf16  [c, feat]; feat-tile order [q01 k01 q23 k23 q45 k45]
  wv   [769, 390]  bf16  [c(+bias row), 6 heads x (ones-col, 64 v-dims)]
  wp   [384, 768]  bf16  [feat (6 heads x 64), out-features]
  bqk  [128, 6]    f32   per-partition bias per qk feature tile
  bp   [128, 6]    f32   b_proj / 2 per out-feature tile
Output:
  yT   [768, 2048] f32   partial (pre pair-sum) transposed projection
"""

import sys
from collections import deque

import numpy as np

if "/opt/trn_rl_repo" not in sys.path:
    sys.path.insert(0, "/opt/trn_rl_repo")

B, P, D = 4, 2048, 768
H, HD = 12, 64
N_CORES = 8
H_LOC = 6
SCALE = HD ** -0.5

CC = 6
FT_COL = {0: 0, 3: 1, 1: 2, 4: 3, 2: 4, 5: 5}  # wqk column-tile by ft
KT = 16
PT = 16
TB = 4
VW = H_LOC * (HD + 1)  # 390
VG = 128
UNIT = 1024
NBLK = 2 * KT          # 32 512-col score blocks per chunk
N_UNITS = NBLK // 2    # 16 units of 1024 per chunk

_PROG = None


def _build_program():
    import concourse.mybir as mybir
    import concourse.tile as tile
    from concourse import bacc

    f32 = mybir.dt.float32
    bf16 = mybir.dt.bfloat16
    AF = mybir.ActivationFunctionType

    nc = bacc.Bacc("TRN2")

    xT = nc.declare_dram_parameter("xT", [769, 2048], bf16, isOutput=False)
    wqk = nc.declare_dram_parameter("wqk", [768, 768], bf16, isOutput=False)
    wv = nc.declare_dram_parameter("wv", [769, VW], bf16, isOutput=False)
    wp = nc.declare_dram_parameter("wp", [384, 768], bf16, isOutput=False)
    bqk = nc.declare_dram_parameter("bqk", [128, 6], f32, isOutput=False)
    bp = nc.declare_dram_parameter("bp", [128, 6], f32, isOutput=False)
    yT = nc.declare_dram_parameter("yT", [768, 2048], f32, isOutput=True)

    with tile.TileContext(nc) as tc:
        with (
            tc.tile_pool(name="persist", bufs=1) as persist,
            tc.tile_pool(name="slabs", bufs=2) as slabs,
            tc.tile_pool(name="norm", bufs=3) as norm,
            tc.tile_pool(name="drs", bufs=4, space="DRAM") as drs,
        ):
            qkt = persist.tile([128, 6, 2048], bf16, tag="qkt")
            vsb = persist.tile([128, KT, H_LOC * VG], bf16, tag="vsb")
            otsb = persist.tile([128, 3, 2048], bf16, tag="otsb")
            bqk_sb = persist.tile([128, 6], f32, tag="bqk_sb")
            bp_sb = persist.tile([128, 6], f32, tag="bp_sb")
            wp_sb = persist.tile([128, 3, 768], bf16, tag="wp_sb")
            xts = [
                persist.tile([128 if i < CC else 1, 2048], bf16,
                             tag=f"xt{i}", name=f"xt{i}")
                for i in range(7)
            ]
            wqk_sbs = [
                persist.tile([128, 768], bf16, tag=f"wqk{i}", name=f"wqk{i}")
                for i in range(CC)
            ]
            wv_sbs = [
                persist.tile([128 if i < CC else 1, VW], bf16,
                             tag=f"wv{i}", name=f"wv{i}")
                for i in range(7)
            ]

            # ---- priority-ordered input DMA (rings are FIFO, run in
            # parallel): sync carries xT then wv then wp; gpsimd carries
            # wqk; scalar only the biases.
            # critical transfers split across BOTH hwdge rings (sync and
            # scalar) so trigger issue is parallel and each ring's share
            # of bandwidth still carries phase-A-critical bytes
            for ccx in range(CC):
                eng = nc.sync if ccx % 2 == 0 else nc.scalar
                eng.dma_start(out=xts[ccx],
                              in_=xT[ccx * 128:(ccx + 1) * 128, :])
                eng.dma_start(
                    out=wqk_sbs[ccx][:, 0:256],
                    in_=wqk[ccx * 128:(ccx + 1) * 128, 0:256])
            nc.scalar.dma_start(out=xts[6], in_=xT[768:769, :])
            nc.scalar.dma_start(out=bqk_sb, in_=bqk[:, :])
            nc.scalar.dma_start(out=bp_sb, in_=bp[:, :])
            for ccx in range(CC):
                nc.sync.dma_start(out=wv_sbs[ccx],
                                  in_=wv[ccx * 128:(ccx + 1) * 128, :])
            nc.sync.dma_start(out=wv_sbs[6], in_=wv[768:769, :])
            for ccx in range(CC):
                nc.sync.dma_start(
                    out=wqk_sbs[ccx][:, 256:768],
                    in_=wqk[ccx * 128:(ccx + 1) * 128, 256:768])
            for fc in range(3):
                nc.sync.dma_start(out=wp_sb[:, fc, :],
                                  in_=wp[fc * 128:(fc + 1) * 128, :])

            # vsb gap-column zeroing on Pool, after its DMA triggers
            nc.gpsimd.memset(
                vsb.rearrange("p a (h g) -> p a h g", g=VG)[:, :, :, 1:64],
                0.0)

            # pre-warm the exp ACT table during the DMA lead
            warmup = norm.tile([1, 1], f32, tag="warmup", bufs=1)
            nc.vector.memset(warmup, 0.0)
            nc.scalar.activation(out=warmup, in_=warmup, func=AF.Exp)
            # PE p-state warm-up during the DMA lead: a few throwaway
            # matmuls keep the tensor engine busy so it ramps before
            # phase A; results land in a phase-A bank and are reset by
            # the group's start=True
            wtile = persist.tile([128, 640], bf16, tag="wtile")
            nc.gpsimd.memset(wtile, 0.0)

            # ===== phase A: ft0 (q pair0) + ft3 (k pair0), cc-outer over
            # 8 PSUM groups so matmuls track the per-cc DMA arrival
            psA_ctx = tc.tile_pool(name="psA", bufs=8, space="PSUM")
            psA = psA_ctx.__enter__()
            qpA = {}
            for ft, tb in ((3, 0), (3, 1), (3, 2), (3, 3), (0, 0)):
                qpA[(ft, tb)] = psA.tile([128, 512], f32, tag="qpA",
                                         name=f"qpA{ft}_{tb}")
            for _ in range(5):
                nc.tensor.matmul(
                    qpA[(3, 0)],
                    wtile[:, 0:128],
                    wtile[:, 128:640],
                    start=True,
                    stop=True,
                )
            for ccx in range(CC):
                for ft, tb in ((3, 0), (3, 1), (3, 2), (3, 3), (0, 0)):
                    fc_ = FT_COL[ft]
                    nc.tensor.matmul(
                        qpA[(ft, tb)],
                        wqk_sbs[ccx][:, fc_ * 128:(fc_ + 1) * 128],
                        xts[ccx][:, tb * 512:(tb + 1) * 512],
                        start=(ccx == 0),
                        stop=(ccx == CC - 1),
                    )
            for ft, tb in ((3, 0), (0, 0), (3, 1), (3, 2), (3, 3)):
                nc.vector.tensor_scalar_add(
                    out=qkt[:, ft, tb * 512:(tb + 1) * 512],
                    in0=qpA[(ft, tb)],
                    scalar1=bqk_sb[:, ft:ft + 1],
                )
            psA_ctx.__exit__(None, None, None)

            # ===== phase B =====
            with (
                tc.tile_pool(name="psum_s", bufs=2, space="PSUM") as psum_s,
                tc.tile_pool(name="psum_o", bufs=2, space="PSUM") as psum_o,
                tc.tile_pool(name="psum_f", bufs=2, space="PSUM") as psum_f,
            ):
                # ---------- fill jobs: fixed-rate stream ----------
                # each fill step is a closure of <=~1.2us PE work; psum_f
                # tiles are held across consecutive steps of one job.
                fills = deque()

                def v_job(pt):
                    """v projection for one token tile: 2 steps"""
                    holder = {}

                    def s1():
                        vp = psum_f.tile([128, 512], f32, tag="fp",
                                         name=f"vp{pt}")
                        holder["vp"] = vp
                        for ccx in range(4):
                            nc.tensor.matmul(
                                vp[:, 0:VW],
                                xts[ccx][:, pt * 128:(pt + 1) * 128],
                                wv_sbs[ccx],
                                start=(ccx == 0),
                                stop=False,
                            )

                    def s2():
                        vp = holder["vp"]
                        for ccx in range(4, 7):
                            kk = 128 if ccx < CC else 1
                            nc.tensor.matmul(
                                vp[:, 0:VW],
                                xts[ccx][0:kk, pt * 128:(pt + 1) * 128],
                                wv_sbs[ccx][0:kk, :],
                                start=False,
                                stop=(ccx == 6),
                            )
                        vpv = vp[:, 0:VW].rearrange("p (h c) -> p h c", c=65)
                        vdst = vsb.rearrange(
                            "p a (h g) -> p a h g", g=VG)[:, pt]
                        nc.vector.tensor_copy(out=vdst[:, :, 0:1],
                                              in_=vpv[:, :, 0:1])
                        nc.vector.tensor_copy(out=vdst[:, :, 64:128],
                                              in_=vpv[:, :, 1:65])

                    return [s1, s2]

                def qk_job(ft, tb):
                    """leftover qk projection feature tile: 2 steps"""
                    holder = {}

                    def s1():
                        qp = psum_f.tile([128, 512], f32, tag="fp",
                                         name=f"qp{ft}_{tb}")
                        holder["qp"] = qp
                        fc_ = FT_COL[ft]
                        for ccx in range(3):
                            nc.tensor.matmul(
                                qp,
                                wqk_sbs[ccx][:, fc_ * 128:(fc_ + 1) * 128],
                                xts[ccx][:, tb * 512:(tb + 1) * 512],
                                start=(ccx == 0),
                                stop=False,
                            )

                    def s2():
                        qp = holder["qp"]
                        fc_ = FT_COL[ft]
                        for ccx in range(3, CC):
                            nc.tensor.matmul(
                                qp,
                                wqk_sbs[ccx][:, fc_ * 128:(fc_ + 1) * 128],
                                xts[ccx][:, tb * 512:(tb + 1) * 512],
                                start=False,
                                stop=(ccx == CC - 1),
                            )
                        nc.vector.tensor_scalar_add(
                            out=qkt[:, ft, tb * 512:(tb + 1) * 512],
                            in0=qp,
                            scalar1=bqk_sb[:, ft:ft + 1],
                        )

                    return [s1, s2]

                def proj_job(of, tb):
                    """output projection tile: 1 step (3 MMs + evac + DMA)"""
                    def s1():
                        pp = psum_f.tile([128, 512], f32, tag="fp",
                                         name=f"pp{of}_{tb}")
                        for fc in range(3):
                            nc.tensor.matmul(
                                pp,
                                wp_sb[:, fc, of * 128:(of + 1) * 128],
                                otsb[:, fc, tb * 512:(tb + 1) * 512],
                                start=(fc == 0),
                                stop=(fc == 2),
                            )
                        ysl = norm.tile([128, 512], f32, tag="ysl")
                        nc.vector.tensor_scalar_add(
                            out=ysl, in0=pp, scalar1=bp_sb[:, of:of + 1])
                        nc.sync.dma_start(
                            out=yT[of * 128:(of + 1) * 128,
                                   tb * 512:(tb + 1) * 512],
                            in_=ysl,
                        )

                    return [s1]

                # fill order: v (needed by AV from chunk 1), then leftover
                # qk (ft1/ft4 by chunk 4, ft2/ft5 by chunk 8); proj jobs
                # are appended when their token block is normalized.
                for tb in range(1, TB):
                    fills.extend(qk_job(0, tb))
                for pt in range(PT):
                    fills.extend(v_job(pt))
                for tb in range(TB):
                    fills.extend(qk_job(4, tb))
                for tb in range(TB):
                    fills.extend(qk_job(1, tb))
                for tb in range(TB):
                    fills.extend(qk_job(5, tb))
                for tb in range(TB):
                    fills.extend(qk_job(2, tb))

                norm_count = {}

                def ot_norm(ph, qq, op):
                    """baseline inline norm: evac + recip + DRAM-bounce
                    broadcast + normalize into otsb"""
                    osb = norm.tile([128, 512], f32, tag="osb")
                    nc.vector.tensor_copy(out=osb, in_=op)
                    rec = norm.tile([1, 512], f32, tag="rec")
                    rsc = norm.tile([1, 512], f32, tag="rsc")
                    nc.vector.reciprocal_approx_accurate(
                        out=rec, in_=osb[0:1, :], scratch=rsc)
                    dsc = drs.tile([1, 512], f32, tag="dsc")
                    nc.sync.dma_start(out=dsc, in_=rec)
                    rb = norm.tile([128, 512], f32, tag="rb")
                    nc.gpsimd.dma_start(out=rb[64:128, :],
                                        in_=dsc.partition_broadcast(64))
                    pb = 64 * (ph % 2)
                    nc.vector.tensor_mul(
                        out=otsb[pb:pb + 64, ph // 2,
                                 qq * 512:(qq + 1) * 512],
                        in0=osb[64:128, :],
                        in1=rb[64:128, :],
                    )
                    norm_count[qq] = norm_count.get(qq, 0) + 1
                    if norm_count[qq] == H_LOC:
                        for of in range(6):
                            fills.extend(proj_job(of, qq))

                def score_mm(p, qq, sp, g, off):
                    kt, hd = g // 2, g % 2
                    pb = 64 * hd
                    qlo = qq * 512
                    nc.tensor.matmul(
                        sp[:, off:off + 512],
                        qkt[pb:pb + 64, 3 + p, kt * 128:(kt + 1) * 128],
                        qkt[pb:pb + 64, p, qlo:qlo + 512],
                        start=True,
                        stop=True,
                    )

                def emit_chunk(cur, prev, n_fill, av_from=0):
                    """16 units: scores+exp for `cur`, AV for `prev`
                    starting at unit av_from; fills in the AV-free units."""
                    ot_jobs = []
                    ot_ps = {}
                    if prev is not None:
                        pp_, pqq, pslab = prev
                        ot_jobs = [(hd, kc) for kc in range(KT)
                                   for hd in range(2)]
                    av_units = N_UNITS - av_from
                    p, qq, slab = cur
                    for u in range(N_UNITS):
                        sp = psum_s.tile([128, UNIT], f32, tag="sp")
                        for j in range(2):
                            score_mm(p, qq, sp, u * 2 + j, j * 512)
                        nc.scalar.activation(
                            out=slab.rearrange("p a b -> p (a b)")[
                                :, u * UNIT:u * UNIT + UNIT],
                            in_=sp,
                            func=AF.Exp,
                            scale=SCALE,
                        )
                        if u >= av_from:
                            left = max(1, min(N_UNITS, av_from + 11) - u)
                            n_do = -(-len(ot_jobs) // left)
                            for _ in range(min(n_do, len(ot_jobs))):
                                hd, kc = ot_jobs.pop(0)
                                if hd not in ot_ps:
                                    ot_ps[hd] = psum_o.tile(
                                        [128, 512], f32, tag="op",
                                        name=f"op{hd}")
                                ph = 2 * pp_ + hd
                                nc.tensor.matmul(
                                    ot_ps[hd],
                                    vsb[:, kc, ph * VG:(ph + 1) * VG],
                                    pslab[:, kc * 2 + hd, :],
                                    start=(kc == 0),
                                    stop=(kc == KT - 1),
                                )
                                if kc == KT - 1:
                                    ot_norm(ph, pqq, ot_ps.pop(hd))
                        else:
                            for _ in range(n_fill + 1):
                                if fills:
                                    fills.popleft()()
                        if (u >= av_from and not ot_jobs
                                and (u >= 11 or prev is None)):
                            extra = 1 if prev is not None else 2
                            for _ in range(extra):
                                if fills:
                                    fills.popleft()()

                prev = None
                for c in range(12):
                    p, qq = c // 4, c % 4
                    slab = slabs.tile([128, NBLK, 512], bf16, tag="slab")
                    nf = 1
                    # chunk 0: no AV, fills everywhere; chunk 1: finish the
                    # v fills in units 0-7, then AV_0 compressed after
                    av_from = 8 if c == 1 else 0
                    emit_chunk((p, qq, slab), prev, nf, av_from)
                    prev = (p, qq, slab)

                # ---- tail: AV of the last chunk + remaining fills ----
                pp_, pqq, pslab = prev
                ot_jobs = [(hd, kc) for hd in range(2) for kc in range(KT)]
                ot_ps = {}
                while ot_jobs:
                    for _ in range(min(4, len(ot_jobs))):
                        hd, kc = ot_jobs.pop(0)
                        if hd not in ot_ps:
                            ot_ps[hd] = psum_o.tile(
                                [128, 512], f32, tag="op", name=f"opf{hd}")
                        ph = 2 * pp_ + hd
                        nc.tensor.matmul(
                            ot_ps[hd],
                            vsb[:, kc, ph * VG:(ph + 1) * VG],
                            pslab[:, kc * 2 + hd, :],
                            start=(kc == 0),
                            stop=(kc == KT - 1),
                        )
                        if kc == KT - 1:
                            ot_norm(ph, pqq, ot_ps.pop(hd))
                    for _ in range(2):
                        if fills:
                            fills.popleft()()
                while fills:
                    fills.popleft()()

    nc.finalize()
    return nc


def _get_program():
    global _PROG
    if _PROG is None:
        _PROG = _build_program()
    return _PROG


def _prep_core_inputs(x, w_qkv, b_qkv, w_proj, b_proj, core):
    b, half = core // 2, core % 2
    heads = np.arange(H_LOC) + H_LOC * half
    d = np.arange(HD)

    import ml_dtypes
    bft = ml_dtypes.bfloat16
    xT = np.empty((769, 2048), bft)
    xT[:768] = x[b].T.astype(bft)
    xT[768] = 1.0

    # torch reshape quirk: feature (t, d, h) -> row t*768 + d*12 + h
    qk_rows = np.empty(768, np.int64)
    for j in range(3):
        for hp in range(2):
            hh = heads[2 * j + hp]
            base = j * 128 + hp * 64
            qk_rows[base:base + 64] = d * 12 + hh
            qk_rows[384 + base:384 + base + 64] = 768 + d * 12 + hh
    bqk = np.ascontiguousarray(b_qkv[qk_rows].reshape(6, 128).T)
    # wqk DRAM column-tile order [ft0 ft3 ft1 ft4 ft2 ft5] so the
    # phase-A-critical ft0/ft3 columns are one contiguous leading slice
    col_order = np.concatenate([np.arange(128) + 128 * ft
                                for ft in (0, 3, 1, 4, 2, 5)])
    wqk = np.ascontiguousarray(w_qkv[qk_rows[col_order]].T.astype(bft))

    wv = np.zeros((769, VW), bft)
    for i in range(H_LOC):
        rows = 1536 + d * 12 + heads[i]
        wv[768, 65 * i] = 1.0
        wv[:768, 65 * i + 1:65 * i + 65] = w_qkv[rows].T.astype(bft)
        wv[768, 65 * i + 1:65 * i + 65] = b_qkv[rows]

    wp = np.empty((384, 768), bft)
    for i in range(H_LOC):
        cols = 64 * heads[i] + d
        wp[64 * i:64 * i + 64] = w_proj[:, cols].T
    bp = np.ascontiguousarray((b_proj * 0.5).reshape(6, 128).T)

    return {
        "xT": xT,
        "wqk": wqk,
        "wv": np.ascontiguousarray(wv),
        "wp": np.ascontiguousarray(wp),
        "bqk": bqk,
        "bp": np.ascontiguousarray(bp),
    }


def _run(inputs, trace=False, **kw):
    from concourse.bass_utils import run_bass_kernel_spmd

    nc = _get_program()
    x = np.asarray(inputs["x"], np.float32)
    w_qkv = np.asarray(inputs["w_qkv"], np.float32)
    b_qkv = np.asarray(inputs["b_qkv"], np.float32)
    w_proj = np.asarray(inputs["w_proj"], np.float32)
    b_proj = np.asarray(inputs["b_proj"], np.float32)

    in_maps = [
        _prep_core_inputs(x, w_qkv, b_qkv, w_proj, b_proj, c)
        for c in range(N_CORES)
    ]
    res = run_bass_kernel_spmd(nc, in_maps, list(range(N_CORES)),
                               trace=trace, **kw)

    out = np.empty((B, P, D), np.float32)
    for b in range(B):
        yt = res.results[2 * b]["yT"] + res.results[2 * b + 1]["yT"]
        out[b] = yt.T
    return out, res


def kernel(**inputs):
    out, _ = _run(inputs)
    return out
